# revision 8
# baseline (speedup 1.0000x reference)
"""Multi-head attention (B=2, S=2048, D=1024, H=16) on 8 NeuronCores.

Sharding: core c handles batch b = c//4 and 4 heads starting at (c%4)*4
(data parallel over batch x tensor parallel over heads; wQ/wK/wV split
column-wise by head, wO row-wise; partial outputs summed on host).

Per-core device program (identical SPMD program, different data):
  phase 1: Q4T/K4T = (w @ x^T) in [head_dim(part), seq(free)] layout,
           V4 = x @ w^T in [seq(part), head_dim(free)] layout with an
           interleaved ones column per head (fused softmax denominator).
  phase 2: per 1024-wide query chunk: scoresT = K_h^T-major QK^T (two heads
           row-packed in the PE array), exp on ScalarE (psum -> sbuf bf16),
           multiplicative {0,1} mask on VectorE (2x bf16 mode), PV matmul
           with the ones column producing row sums, reciprocal + broadcast
           DMA + multiply to normalize.
  phase 3: out_partial = ctx^T @ wO_cols (K=128 chunks), DMA to DRAM.

Host: out[b] = sum of the 4 cores' partials + wO_b.
"""

import numpy as np
from contextlib import ExitStack

import concourse.bacc as bacc
import concourse.tile as tile
from concourse import mybir
from concourse.bass_utils import run_bass_kernel_spmd
import ml_dtypes

B, S, DM, H, DK = 2, 2048, 1024, 16, 64
NCORES = 8
GROUPS = 4          # cores per batch
HPC = H // GROUPS   # heads per core = 4
P = 128
KT = DM // P        # 8 k-tiles over the model dim
CW = HPC * DK       # projected width per core = 256
SCALE = 1.0 / np.sqrt(DK)

# dtype for exp/probabilities + mask (bf16: DVE 2x mode, half DMA/SBUF)
EXP_DT = mybir.dt.bfloat16
EXP_NP = ml_dtypes.bfloat16

F32 = mybir.dt.float32
EXPF = mybir.ActivationFunctionType.Exp

_cache: dict = {}


def _build():
    nc = bacc.Bacc("TRN2", target_bir_lowering=False, debug=False)

    xqT = nc.dram_tensor("xqT", [DM, S], F32, kind="ExternalInput")
    xkT = nc.dram_tensor("xkT", [DM, S], F32, kind="ExternalInput")
    xvT = nc.dram_tensor("xvT", [DM, S], F32, kind="ExternalInput")
    wqT = nc.dram_tensor("wqT", [DM, CW], F32, kind="ExternalInput")
    wkT = nc.dram_tensor("wkT", [DM, CW], F32, kind="ExternalInput")
    wvT = nc.dram_tensor("wvT", [DM, CW], F32, kind="ExternalInput")
    wqb = nc.dram_tensor("wqb", [CW, 1], F32, kind="ExternalInput")
    wkb = nc.dram_tensor("wkb", [CW, 1], F32, kind="ExternalInput")
    wvb = nc.dram_tensor("wvb", [1, CW], F32, kind="ExternalInput")
    woT = nc.dram_tensor("woT", [CW, DM], F32, kind="ExternalInput")
    maskT = nc.dram_tensor("maskT", [S, S], EXP_DT, kind="ExternalInput")
    out = nc.dram_tensor("out", [S, DM], F32, kind="ExternalOutput")

    with tile.TileContext(nc) as tc, ExitStack() as ctx:
        const = ctx.enter_context(tc.tile_pool(name="const", bufs=1))
        wo_pool = ctx.enter_context(tc.tile_pool(name="wo_pool", bufs=1))
        big = ctx.enter_context(tc.tile_pool(name="big", bufs=1))

        ones_row = const.tile([1, 512], F32)
        nc.vector.memset(ones_row[:], 1.0)

        # persistent activations
        Q4T = big.tile([P, 2, S], EXP_DT, name="Q4T")      # [hd%128, pair, s]
        K4T = big.tile([P, 2, S], EXP_DT, name="K4T")
        V4x = big.tile([P, 16, HPC * (DK + 1)], EXP_DT, name="V4x")  # ones col per head
        ctxT = [big.tile([P, S], F32, name=f"ctxT{i}") for i in range(2)]

        woT_sb = wo_pool.tile([P, 2, DM], F32)
        nc.sync.dma_start(out=woT_sb[:], in_=woT.ap().rearrange("(c p) n -> p c n", p=P))

        # ones columns of V4x (col h*65+64 = 1.0)
        for h in range(HPC):
            nc.vector.memset(V4x[:, :, h * 65 + 64 : h * 65 + 65], 1.0)

        # ---------------- phase 1: projections ----------------
        with tc.tile_pool(name="wpool", bufs=1) as wpool, \
             tc.tile_pool(name="xpool", bufs=2) as xpool, \
             tc.tile_pool(name="pp", bufs=3, space="PSUM") as pp:
            w_sbs = {}
            b_sbs = {}
            for nm, wd, bd in (("q", wqT, wqb), ("k", wkT, wkb), ("v", wvT, wvb)):
                w_sb = wpool.tile([P, KT, CW], F32, name=f"w{nm}_sb")
                nc.sync.dma_start(out=w_sb[:], in_=wd.ap().rearrange("(t p) m -> p t m", p=P))
                if nm == "v":
                    b_sb = wpool.tile([1, CW], F32, name=f"b{nm}_sb")
                    nc.sync.dma_start(out=b_sb[:], in_=bd.ap())
                else:
                    b_sb = wpool.tile([P, 2], F32, name=f"b{nm}_sb")
                    nc.sync.dma_start(out=b_sb[:], in_=bd.ap().rearrange("(m p) o -> p (m o)", p=P))
                w_sbs[nm], b_sbs[nm] = w_sb, b_sb

            for nm, xd, dstQK in (("q", xqT, Q4T), ("k", xkT, K4T), ("v", xvT, None)):
                w_sb, b_sb = w_sbs[nm], b_sbs[nm]
                for sc in range(4):
                    x_sb = xpool.tile([P, KT, 512], F32, tag="x")
                    nc.sync.dma_start(
                        out=x_sb[:],
                        in_=xd.ap().rearrange("(t p) s -> p t s", p=P)[:, :, sc * 512:(sc + 1) * 512],
                    )
                    if dstQK is not None:
                        # out tile = [head_dim 128, s 512]
                        for m in range(2):
                            ps = pp.tile([P, 512], F32, tag="ps")
                            for kt in range(KT):
                                nc.tensor.matmul(
                                    ps[:], w_sb[:, kt, m * 128:(m + 1) * 128],
                                    x_sb[:, kt, :], start=(kt == 0), stop=(kt == KT - 1))
                            nc.scalar.activation(
                                dstQK[:, m, sc * 512:(sc + 1) * 512], ps[:],
                                mybir.ActivationFunctionType.Identity,
                                bias=b_sb[:, m:m + 1])
                    else:
                        # V: out tile = [s 128, head_dim 256]
                        for st in range(4):
                            ps = pp.tile([P, 512], F32, tag="ps")
                            for kt in range(KT):
                                nc.tensor.matmul(
                                    ps[:, 0:CW], x_sb[:, kt, st * 128:(st + 1) * 128],
                                    w_sb[:, kt, :], start=(kt == 0), stop=False)
                            nc.tensor.matmul(
                                ps[:, 0:CW], ones_row[0:1, 0:128],
                                b_sb[0:1, :], start=False, stop=True)
                            sidx = sc * 4 + st
                            nc.vector.tensor_copy(
                                out=V4x.rearrange("p s (h e) -> p s h e", e=DK + 1)[:, sidx, :, 0:DK],
                                in_=ps[:, 0:CW].rearrange("p (h e) -> p h e", e=DK))

        # ---------------- phase 2: attention ----------------
        with tc.tile_pool(name="mpool", bufs=1) as mpool, \
             tc.tile_pool(name="epool", bufs=17) as epool, \
             tc.tile_pool(name="rpool", bufs=2) as rpool, \
             tc.tile_pool(name="ps_sc", bufs=2, space="PSUM") as ps_sc, \
             tc.tile_pool(name="ps_ctx", bufs=2, space="PSUM") as ps_ctx:
            for s1c in range(2):
                mask_sb = mpool.tile([P, 16, 1024], EXP_DT, tag="mask")
                nc.sync.dma_start(
                    out=mask_sb[:],
                    in_=maskT.ap().rearrange("(t p) s -> p t s", p=P)[:, :, s1c * 1024:(s1c + 1) * 1024],
                )
                for hp in range(2):
                    etiles = ([], [])
                    for s2t in range(16):
                        pss = []
                        for hh in range(2):
                            ps = ps_sc.tile([P, 1024], F32, tag="sc")
                            lhsT = K4T[hh * 64:(hh + 1) * 64, hp, s2t * 128:(s2t + 1) * 128]
                            for n2 in range(2):
                                col = s1c * 1024 + n2 * 512
                                nc.tensor.matmul(
                                    ps[:, n2 * 512:(n2 + 1) * 512], lhsT,
                                    Q4T[hh * 64:(hh + 1) * 64, hp, col:col + 512],
                                    start=True, stop=True)
                            pss.append(ps)
                        for hh in range(2):
                            et = epool.tile([P, 1024], EXP_DT, tag=f"e{hh}")
                            nc.scalar.activation(et[:], pss[hh][:], EXPF)
                            nc.vector.tensor_mul(et[:], et[:], mask_sb[:, s2t, :])
                            etiles[hh].append(et)
                    for hh in range(2):
                        h = hp * 2 + hh
                        cps = ps_ctx.tile([DK + 1, 1024], F32, tag="ctx")
                        for nh in range(2):
                            for s2t in range(16):
                                nc.tensor.matmul(
                                    cps[:, nh * 512:(nh + 1) * 512],
                                    V4x[:, s2t, h * 65:(h + 1) * 65],
                                    etiles[hh][s2t][:, nh * 512:(nh + 1) * 512],
                                    start=(s2t == 0), stop=(s2t == 15))
                        rr = rpool.tile([DK + 1, 1024], F32, tag="rr")
                        nc.vector.tensor_copy(out=rr[64:65, :], in_=cps[64:65, :])
                        rr0 = rpool.tile([1, 1024], F32, tag="rr0")
                        nc.sync.dma_start(out=rr0[:], in_=rr[64:65, :])
                        rc = rpool.tile([1, 1024], F32, tag="rc")
                        nc.vector.reciprocal_approx_fast(out=rc[0:1, :], in_=rr0[0:1, :])
                        bc = rpool.tile([64, 1024], F32, tag="bc")
                        nc.gpsimd.partition_broadcast(bc[:], rc[0:1, :])
                        col = s1c * 1024
                        if hh == 0:
                            nc.vector.tensor_mul(
                                ctxT[hp][0:64, col:col + 1024], cps[0:64, :], bc[:])
                        else:
                            ht = rpool.tile([64, 1024], F32, tag="rr")
                            nc.vector.tensor_mul(ht[:], cps[0:64, :], bc[:])
                            nc.sync.dma_start(
                                out=ctxT[hp][64:128, col:col + 1024], in_=ht[:])

        # ---------------- phase 3: output projection ----------------
        with tc.tile_pool(name="opool", bufs=3) as opool, \
             tc.tile_pool(name="ps_o", bufs=2, space="PSUM") as ps_o:
            for s1t in range(16):
                ob = opool.tile([P, DM], F32, tag="ob")
                for n2 in range(2):
                    ps = ps_o.tile([P, 512], F32, tag="po")
                    for c2 in range(2):
                        nc.tensor.matmul(
                            ps[:], ctxT[c2][:, s1t * 128:(s1t + 1) * 128],
                            woT_sb[:, c2, n2 * 512:(n2 + 1) * 512],
                            start=(c2 == 0), stop=(c2 == 1))
                    nc.vector.tensor_copy(out=ob[:, n2 * 512:(n2 + 1) * 512], in_=ps[:])
                nc.sync.dma_start(out=out.ap()[s1t * 128:(s1t + 1) * 128, :], in_=ob[:])

    nc.compile()
    return nc


def get_nc():
    if "nc" not in _cache:
        _cache["nc"] = _build()
    return _cache["nc"]


def make_in_maps(q, k, v, mask, wQ_w, wQ_b, wK_w, wK_b, wV_w, wV_b, wO_w, wO_b):
    q = np.asarray(q, np.float32)
    k = np.asarray(k, np.float32)
    v = np.asarray(v, np.float32)
    mask = np.asarray(mask)
    qT = np.ascontiguousarray(q.transpose(0, 2, 1))
    kT = np.ascontiguousarray(k.transpose(0, 2, 1))
    vT = np.ascontiguousarray(v.transpose(0, 2, 1))
    mT = np.ascontiguousarray(mask[:, 0].transpose(0, 2, 1)).astype(EXP_NP)
    in_maps = []
    for c in range(NCORES):
        b = c // GROUPS
        rows = slice((c % GROUPS) * HPC * DK, ((c % GROUPS) + 1) * HPC * DK)
        in_maps.append({
            "xqT": qT[b], "xkT": kT[b], "xvT": vT[b],
            "wqT": np.ascontiguousarray(np.asarray(wQ_w, np.float32)[rows].T) * np.float32(SCALE),
            "wkT": np.ascontiguousarray(np.asarray(wK_w, np.float32)[rows].T),
            "wvT": np.ascontiguousarray(np.asarray(wV_w, np.float32)[rows].T),
            "wqb": (np.asarray(wQ_b, np.float32)[rows] * np.float32(SCALE)).reshape(-1, 1),
            "wkb": np.asarray(wK_b, np.float32)[rows].reshape(-1, 1),
            "wvb": np.asarray(wV_b, np.float32)[rows].reshape(1, -1),
            "woT": np.ascontiguousarray(np.asarray(wO_w, np.float32)[:, rows].T),
            "maskT": mT[b],
        })
    return in_maps


def kernel(q, k, v, mask, wQ_w, wQ_b, wK_w, wK_b, wV_w, wV_b, wO_w, wO_b):
    nc = get_nc()
    in_maps = make_in_maps(q, k, v, mask, wQ_w, wQ_b, wK_w, wK_b, wV_w, wV_b,
                           wO_w, wO_b)
    res = run_bass_kernel_spmd(nc, in_maps, core_ids=list(range(NCORES)))
    outs = [res.results[c]["out"] for c in range(NCORES)]
    ob = np.asarray(wO_b, np.float32)
    full = np.empty((B, S, DM), np.float32)
    for b in range(B):
        acc = outs[b * GROUPS].astype(np.float32)
        for g in range(1, GROUPS):
            acc = acc + outs[b * GROUPS + g]
        full[b] = acc + ob[None, :]
    return full
